# revision 2
# baseline (speedup 1.0000x reference)
"""Trainium2 Bass kernel for nn_Block_420906795461 (dense transformer block).

Data-parallel over B=8 clip-pairs across 8 NeuronCores; each core runs the
full block on its [2, 512, 2048] shard with replicated weights; no
collectives.

The attention path (QKV, softmax-denominator, attn-output, proj) runs in
fp8(e4m3) DoubleRow perf mode: each PE instruction consumes TWO 128-deep
contraction slices (lhsT [128,2,M], rhs [128,2,N]), doubling matmul
throughput vs bf16 (validated 2.0x on HW). The MLP stays bf16: fp8 noise
on either MLP matmul alone costs ~3e-2 relative error vs the 2e-2 gate
(attention-path fp8 costs only ~2e-3 extra because the attention branch
is small relative to the residual stream). Attention scores stay bf16
(single 128-deep head-dim contraction can't pair into DoubleRow).

Scaling (powers of two, exact): fp8 weights x512 host-side; LN1 output
x16 (folded into gamma1/beta1 host-side); attention output x32; softmax
runs as exp(SCALE*s - 2) with the common e^-2 cancelling in the
normalizer. Every descale is folded into an existing PSUM-evacuation op.

Engine balance: LayerNorm Square and gamma/beta-apply run on DVE (not
ACT) so the ACT engine only holds exp/gelu; attention scores+exp are
interleaved with the v-projection groups so the 32 softmax exp passes
(~2us each on ACT) hide under v's tensor work; attention output o stays
in SBUF (no DRAM staging).
"""

import sys

import numpy as np

sys.path.insert(0, "/opt/trn_rl_repo")

from contextlib import ExitStack

import concourse.bass as bass  # noqa: F401
import concourse.mybir as mybir
import concourse.tile as tile
from concourse import bacc
from concourse.bass_utils import run_bass_kernel_spmd

FP32 = mybir.dt.float32
FP32R = mybir.dt.float32r
BF16 = mybir.dt.bfloat16
FP8 = mybir.dt.float8e4
AF = mybir.ActivationFunctionType
ALU = mybir.AluOpType
DR = mybir.MatmulPerfMode.DoubleRow

DIM = 2048
HEADS = 16
HD = 128
F = 4 * DIM          # 8192
TOK = 1024           # tokens per core (2 clips x 512)
NH = 512             # tokens per clip
CT = DIM // 128      # 16 c-tiles
KP = DIM // 256      # 8 contraction pairs
FT = F // 128        # 64 f-tiles
CH = 16              # mlp chunk size in f-tiles
SCALE = HD ** -0.5
EPS = 1e-5
N_CORES = 8

S_W = 512.0          # fp8 weight scale (host-side)
S_X = 16.0           # LN1 output scale (folded into gamma1/beta1)
S_O = 32.0           # attention-output scale
EXP_BIAS = -2.0      # exp(SCALE*s + EXP_BIAS); cancels in softmax ratio
EV_KQ = 1.0 / (S_W * S_X)          # k/q psum -> natural bf16
EV_V = S_X / (S_W * S_X)           # v psum -> fp8 at x16
EV_O = S_O / S_X                   # o psum * r_b -> fp8 at x32
EV_PJ = 1.0 / (S_W * S_O)          # proj psum -> natural fp32


def _patch_act_tables():
    """Force the ACT-table chooser onto two sets that jointly cover every
    activation function this kernel uses ({Ln, Exp, Copy} pre-MLP, + Gelu),
    so the table is swapped once instead of several times (each mid-kernel
    ACT_TABLE_LOAD is a 1.28us stall in a serial chain)."""
    import concourse.bacc as _bacc

    _orig = _bacc.get_activation_tables
    keep = {"natural_log_exp_and_others", "gelu_and_others"}

    def patched(arch):
        return {name: (funcs if name in keep else set())
                for name, funcs in _orig(arch).items()}

    _bacc.get_activation_tables = patched
    return _orig


def build():
    nc = bacc.Bacc("TRN2", target_bir_lowering=False, debug=False)

    # xrT = x.T + bproj (residual source, pre-biased host-side)
    xrT = nc.dram_tensor("xrT", [DIM, TOK], FP32, kind="ExternalInput").ap()
    xTbf = nc.dram_tensor("xTbf", [DIM, TOK], BF16,
                          kind="ExternalInput").ap()
    # fp8 weights, x512, DoubleRow-interleaved (see _prep_shared)
    wq8 = nc.dram_tensor("wq8", [128, 2 * KP * 2 * 1024], FP8,
                         kind="ExternalInput").ap()
    wk8 = nc.dram_tensor("wk8", [128, 4 * KP * 2 * 512], FP8,
                         kind="ExternalInput").ap()
    wv8 = nc.dram_tensor("wv8", [128, 4 * KP * 2 * 512], FP8,
                         kind="ExternalInput").ap()
    wp8 = nc.dram_tensor("wp8", [128, 4 * KP * 2 * 512], FP8,
                         kind="ExternalInput").ap()
    # bf16 MLP weights (pre-transposed)
    w1T = nc.dram_tensor("w1T", [DIM, F], BF16, kind="ExternalInput").ap()
    w2T = nc.dram_tensor("w2T", [F, DIM], BF16, kind="ExternalInput").ap()
    g1v = nc.dram_tensor("g1v", [128, CT], FP32, kind="ExternalInput").ap()
    be1v = nc.dram_tensor("be1v", [128, CT], FP32, kind="ExternalInput").ap()
    g2v = nc.dram_tensor("g2v", [128, CT], FP32, kind="ExternalInput").ap()
    be2v = nc.dram_tensor("be2v", [128, CT], FP32, kind="ExternalInput").ap()
    b1v = nc.dram_tensor("b1v", [128, FT], FP32, kind="ExternalInput").ap()
    b2v = nc.dram_tensor("b2v", [128, CT], FP32, kind="ExternalInput").ap()
    outT = nc.dram_tensor("out", [DIM, TOK], FP32, kind="ExternalOutput").ap()

    with tile.TileContext(nc, pool_alloc_mode="stack") as tc, \
            ExitStack() as top:
        consts = top.enter_context(tc.tile_pool(name="consts", bufs=1))

        onesm_bf = consts.tile([128, 128], BF16, tag="onesmbf")
        nc.vector.memset(onesm_bf, 1.0)
        onesm_f = consts.tile([128, 128], FP32, tag="onesmf")
        nc.vector.memset(onesm_f, 1.0)
        onesm_r = onesm_f.bitcast(FP32R)
        ones8 = consts.tile([128, 2, 128], FP8, tag="ones8")
        nc.vector.memset(ones8, 1.0)
        eps128 = consts.tile([128, 1], FP32, tag="eps")
        nc.vector.memset(eps128, EPS)
        ebias = consts.tile([128, 1], FP32, tag="ebias")
        nc.vector.memset(ebias, EXP_BIAS)
        # dummy ACT op: triggers the (single) pre-MLP ACT table load during
        # the initial input DMAs instead of stalling the first Ln
        warm = consts.tile([128, 1], FP32, tag="warm")
        nc.scalar.activation(out=warm, in_=eps128, func=AF.Exp)

        def load_const(name, src, cols):
            t = consts.tile([128, cols], FP32, tag=name)
            nc.sync.dma_start(out=t, in_=src)
            return t

        g1s = load_const("g1s", g1v, CT)
        be1s = load_const("be1s", be1v, CT)
        g2s = load_const("g2s", g2v, CT)
        be2s = load_const("be2s", be2v, CT)
        b1s = load_const("b1s", b1v, FT)
        b2s = load_const("b2s", b2v, CT)

        def layernorm_wide(uid, loader, g_s, be_s, out_writer,
                           in_fp32r=False, pools=None):
            """Full-width (TOK) LN. loader(ct, pass_i) returns a [128, TOK]
            AP: bf16 normally, fp32r when in_fp32r. out_writer(ct, j)
            returns the [128, NH] destination AP for clip j's normalized
            output. The normalize pass emits ALL of clip0 before clip1 so
            clip0-only consumers (q projection, W1's nh=0 groups) start
            ~1us after the stats chain. pools=(ln_ps, sqp, tmpp, vecp)
            reuses pre-opened pools so the stats can overlap a preceding
            phase."""
            with ExitStack() as ln:
                if pools is not None:
                    ln_ps, sqp, tmpp, vecp = pools
                else:
                    ln_ps = ln.enter_context(
                        tc.tile_pool(name=f"lnps{uid}", bufs=4, space="PSUM"))
                    sqp = ln.enter_context(
                        tc.tile_pool(name=f"sq{uid}", bufs=3))
                    tmpp = ln.enter_context(
                        tc.tile_pool(name=f"tm{uid}", bufs=3))
                    vecp = ln.enter_context(
                        tc.tile_pool(name=f"ve{uid}", bufs=4))

                ones_st = onesm_r if in_fp32r else onesm_bf
                sq_dt = FP32R if in_fp32r else BF16
                ps_tag = "pj" if pools is not None else "ln"
                ps_sum = [ln_ps.tile([128, NH], FP32, tag=ps_tag,
                                     name=f"lns{uid}{i}") for i in range(2)]
                ps_sq = [ln_ps.tile([128, NH], FP32, tag=ps_tag,
                                    name=f"lnq{uid}{i}") for i in range(2)]
                for ct in range(CT):
                    xin = loader(ct, 0)
                    sq = sqp.tile([128, TOK], sq_dt, tag="sq")
                    if in_fp32r:
                        # fp32r stats input must be produced rounded-to-fp32r
                        # (BIR verifier), so the square stays on ACT here
                        nc.scalar.activation(out=sq, in_=xin.bitcast(FP32),
                                             func=AF.Square)
                    else:
                        nc.vector.tensor_mul(out=sq, in0=xin, in1=xin)
                    sq_mm = sq
                    for i in range(2):
                        sl = slice(i * NH, (i + 1) * NH)
                        nc.tensor.matmul(out=ps_sum[i], lhsT=ones_st,
                                         rhs=xin[:, sl],
                                         start=(ct == 0), stop=(ct == CT - 1))
                        nc.tensor.matmul(out=ps_sq[i], lhsT=ones_st,
                                         rhs=sq_mm[:, sl],
                                         start=(ct == 0), stop=(ct == CT - 1))
                mean_b = vecp.tile([128, TOK], FP32, tag="v")
                ex2 = vecp.tile([128, TOK], FP32, tag="v")
                for i in range(2):
                    sl = slice(i * NH, (i + 1) * NH)
                    nc.vector.tensor_scalar_mul(out=mean_b[:, sl],
                                                in0=ps_sum[i],
                                                scalar1=1.0 / DIM)
                    nc.vector.tensor_scalar_mul(out=ex2[:, sl],
                                                in0=ps_sq[i],
                                                scalar1=1.0 / DIM)
                msq = vecp.tile([128, TOK], FP32, tag="v")
                nc.vector.tensor_mul(out=msq, in0=mean_b, in1=mean_b)
                var = vecp.tile([128, TOK], FP32, tag="v")
                nc.vector.tensor_sub(out=var, in0=ex2, in1=msq)
                # rstd = (var+eps)^-0.5 = exp(-0.5*ln(var+eps)); keeps the
                # whole pre-MLP kernel on one ACT table (no Sqrt swap-ins)
                lnv = vecp.tile([128, TOK], FP32, tag="v")
                nc.scalar.activation(out=lnv, in_=var, func=AF.Ln,
                                     bias=eps128)
                rstd_b = vecp.tile([128, TOK], FP32, tag="v")
                nc.scalar.activation(out=rstd_b, in_=lnv, func=AF.Exp,
                                     scale=-0.5)
                for j in range(2):
                    sl = slice(j * NH, (j + 1) * NH)
                    for ct in range(CT):
                        xin = loader(ct, 1)
                        ap_in = xin.bitcast(FP32) if in_fp32r else xin
                        t1 = tmpp.tile([128, NH], BF16, tag="t1")
                        nc.vector.tensor_sub(out=t1, in0=ap_in[:, sl],
                                             in1=mean_b[:, sl])
                        t2 = tmpp.tile([128, NH], BF16, tag="t2")
                        nc.vector.tensor_mul(out=t2, in0=t1,
                                             in1=rstd_b[:, sl])
                        # gamma/beta on ACT (idle here)
                        nc.scalar.activation(out=out_writer(ct, j), in_=t2,
                                             func=AF.Identity,
                                             scale=g_s[:, ct:ct + 1],
                                             bias=be_s[:, ct:ct + 1])

        # right-side persistent pools
        pjs_stack = ExitStack()
        wp_pool = pjs_stack.enter_context(
            tc.tile_pool(name="wp", bufs=4, side="right"))
        xr_pool = pjs_stack.enter_context(
            tc.tile_pool(name="xr", bufs=6, side="right"))
        osb_pool = pjs_stack.enter_context(
            tc.tile_pool(name="osb", bufs=1, side="right"))
        v_stack = ExitStack()
        v_pool = v_stack.enter_context(
            tc.tile_pool(name="vp", bufs=1, side="right"))

        # o kept on-chip: pair tiles [128, 2, TOK], heads (2g, 2g+1)
        o_sb = [osb_pool.tile([128, 2, TOK], FP8, tag=f"osb{g}",
                              name=f"osb{g}") for g in range(KP)]
        # v pair tiles [128 tok, 2, DIM]: dim1 = token-tile parity
        v_tiles = {j: [v_pool.tile([128, 2, DIM], FP8, tag=f"v{j}_{u}",
                                   name=f"vt{j}_{u}") for u in range(2)]
                   for j in range(2)}

        # ================= LN1 (both clips, full width) =================
        xt_stack = ExitStack()
        xtp = xt_stack.enter_context(
            tc.tile_pool(name="xt", bufs=1, side="right"))
        xt = [xtp.tile([128, 2, TOK], FP8, tag=f"xt{c}", name=f"xtp{c}")
              for c in range(KP)]
        # QKV weight pool opened BEFORE LN1's transient pools: its SBUF
        # region must not overlap theirs, else the k-weight DMAs stall
        # until the whole LN1 normalize drains (measured 39us PE gap)
        qkw_stack = ExitStack()
        wt_pool = qkw_stack.enter_context(tc.tile_pool(name="wqkv", bufs=10))
        with ExitStack() as ph:
            # all 16 x tiles stay resident (32KB/part; SBUF is empty here):
            # one DMA pass over 4 queues instead of two passes over 2
            xep = ph.enter_context(tc.tile_pool(name="xe", bufs=16))
            x_cache = {}

            def x_loader(ct, pass_i, _xep=xep):
                if ct in x_cache:
                    return x_cache[ct]
                t = _xep.tile([128, TOK], BF16, tag="xe", name=f"xe{ct}")
                eng = (nc.sync, nc.gpsimd, nc.scalar)[ct % 3]
                eng.dma_start(out=t, in_=xTbf[ct * 128:(ct + 1) * 128, :])
                x_cache[ct] = t
                return t

            layernorm_wide("l1", x_loader, g1s, be1s,
                           lambda ct, j: xt[ct // 2][:, ct % 2,
                                            j * NH:(j + 1) * NH])

        # ================= k, q (fp8 DoubleRow) =================
        kq_stack = ExitStack()
        k_pool = kq_stack.enter_context(tc.tile_pool(name="kp", bufs=1))
        q_pool = kq_stack.enter_context(tc.tile_pool(name="qT", bufs=1))
        k_tiles = {0: {}, 1: {}}
        q_tiles = {}
        with ExitStack() as qk:
            qkv_ps = qk.enter_context(
                tc.tile_pool(name="qkvps", bufs=8, space="PSUM"))

            # q (clip0 only): og-pair batched weight loads
            for ogp in range(2):
                pss = [qkv_ps.tile([128, NH], FP32, tag="qkv",
                                   name=f"psq{ogp}_{i}") for i in range(8)]
                for kp in range(KP):
                    wt = wt_pool.tile([128, 2, 1024], FP8, tag="wq")
                    c0 = (ogp * KP + kp) * 2048
                    nc.sync.dma_start(out=wt, in_=wq8[:, c0:c0 + 2048])
                    for i in range(8):
                        nc.tensor.matmul(
                            out=pss[i],
                            lhsT=wt[:, :, i * 128:(i + 1) * 128],
                            rhs=xt[kp][:, :, 0:NH],
                            perf_mode=DR,
                            start=(kp == 0), stop=(kp == KP - 1))
                for i in range(8):
                    go = ogp * 8 + i
                    qt = q_pool.tile([128, NH], BF16, tag=f"q{go}",
                                     name=f"qt{go}")
                    nc.any.tensor_scalar_mul(out=qt, in0=pss[i],
                                             scalar1=EV_KQ)
                    q_tiles[go] = qt

            # k: for each og, 8 accumulators (4 o-tiles x 2 clips)
            for og in range(4):
                pss = {}
                for ot in range(4):
                    for j in range(2):
                        pss[(ot, j)] = qkv_ps.tile(
                            [128, NH], FP32, tag="qkv",
                            name=f"psk{og}_{ot}_{j}")
                for kp in range(KP):
                    wt = wt_pool.tile([128, 2, 512], FP8, tag="w")
                    c0 = (og * KP + kp) * 1024
                    nc.sync.dma_start(out=wt, in_=wk8[:, c0:c0 + 1024])
                    for ot in range(4):
                        for j in range(2):
                            nc.tensor.matmul(
                                out=pss[(ot, j)],
                                lhsT=wt[:, :, ot * 128:(ot + 1) * 128],
                                rhs=xt[kp][:, :, j * NH:(j + 1) * NH],
                                perf_mode=DR,
                                start=(kp == 0), stop=(kp == KP - 1))
                for ot in range(4):
                    go = og * 4 + ot
                    for j in range(2):
                        kt = k_pool.tile([128, NH], BF16, tag=f"k{j}_{go}",
                                         name=f"kt{j}_{go}")
                        nc.any.tensor_scalar_mul(out=kt, in0=pss[(ot, j)],
                                                 scalar1=EV_KQ)
                        k_tiles[j][go] = kt

        # ====== attention: scores+exp / v / denom+o interleaved ======
        # group g: scores/exp for heads 4g..4g+3 (both clips), then the
        # v-projection for feature block vg=g (exp ACT passes hide under
        # v's tensor work), then denominator + output for those heads
        # (their exp tiles free immediately after).
        with ExitStack() as sv:
            s_ps = sv.enter_context(
                tc.tile_pool(name="sps", bufs=2, space="PSUM"))
            v_ps = sv.enter_context(
                tc.tile_pool(name="vps", bufs=4, space="PSUM"))
            sum_ps = sv.enter_context(
                tc.tile_pool(name="sums", bufs=1, space="PSUM"))
            o_ps = sv.enter_context(
                tc.tile_pool(name="ops", bufs=1, space="PSUM"))
            wv_pool = sv.enter_context(tc.tile_pool(name="wv", bufs=10))
            e_pool = sv.enter_context(tc.tile_pool(name="ex", bufs=20))
            bcp = sv.enter_context(tc.tile_pool(name="ab", bufs=3))
            for g in range(4):
                e_tiles = {}
                for h in range(4 * g, 4 * g + 4):
                    qh = q_tiles[h]
                    for j in range(2):
                        ep = [e_pool.tile([128, 2, NH], FP8, tag="e",
                                          name=f"e{h}_{j}_{u}")
                              for u in range(2)]
                        e_tiles[(h, j)] = ep
                        for mt in range(4):
                            ps_s = s_ps.tile([128, NH], FP32, tag="s")
                            nc.tensor.matmul(
                                out=ps_s,
                                lhsT=k_tiles[j][h][:,
                                                   mt * 128:(mt + 1) * 128],
                                rhs=qh, start=True, stop=True)
                            nc.scalar.activation(out=ep[mt // 2][:, mt % 2, :],
                                                 in_=ps_s, func=AF.Exp,
                                                 scale=SCALE, bias=ebias)
                # v for vg=g, one clip at a time (4 PSUM banks each)
                wvts = []
                for j in range(2):
                    psv = [v_ps.tile([128, 512], FP32, tag="v",
                                     name=f"psv{g}_{j}_{tt}")
                           for tt in range(4)]
                    for kp in range(KP):
                        if j == 0:
                            wt = wv_pool.tile([128, 2, 512], FP8, tag="w")
                            c0 = (g * KP + kp) * 1024
                            nc.sync.dma_start(out=wt,
                                              in_=wv8[:, c0:c0 + 1024])
                            wvts.append(wt)
                        else:
                            wt = wvts[kp]
                        for tt in range(4):
                            t0 = j * NH + tt * 128
                            nc.tensor.matmul(
                                out=psv[tt],
                                lhsT=xt[kp][:, :, t0:t0 + 128],
                                rhs=wt,
                                perf_mode=DR,
                                start=(kp == 0), stop=(kp == KP - 1))
                    for tt in range(4):
                        nc.any.tensor_scalar_mul(
                            out=v_tiles[j][tt // 2][:, tt % 2,
                                                    g * 512:(g + 1) * 512],
                            in0=psv[tt], scalar1=EV_V)
                # denominator + attention output for this group's heads
                for h in range(4 * g, 4 * g + 4):
                    for j in range(2):
                        c0 = j * NH
                        ep = e_tiles[(h, j)]
                        ps_sum = sum_ps.tile([128, NH], FP32, tag="as")
                        for u in range(2):
                            nc.tensor.matmul(out=ps_sum, lhsT=ones8,
                                             rhs=ep[u], perf_mode=DR,
                                             start=(u == 0), stop=(u == 1))
                        r_b = bcp.tile([128, NH], FP32, tag="rb")
                        nc.vector.reciprocal_approx_fast(out=r_b, in_=ps_sum)
                        ps_o = o_ps.tile([128, NH], FP32, tag="o")
                        for u in range(2):
                            nc.tensor.matmul(
                                out=ps_o,
                                lhsT=v_tiles[j][u][:, :,
                                                   h * 128:(h + 1) * 128],
                                rhs=ep[u], perf_mode=DR,
                                start=(u == 0), stop=(u == 1))
                        nc.vector.scalar_tensor_tensor(
                            out=o_sb[h // 2][:, h % 2, c0:c0 + NH],
                            in0=ps_o, scalar=EV_O, in1=r_b,
                            op0=ALU.mult, op1=ALU.mult)
        kq_stack.close()
        qkw_stack.close()
        xt_stack.close()
        v_stack.close()

        # ================= Projection + residual =================
        xmid_stack = ExitStack()
        xm_pool = xmid_stack.enter_context(tc.tile_pool(name="xmid", bufs=1))
        xt2_stack = ExitStack()
        xt2_pool = xt2_stack.enter_context(tc.tile_pool(name="xt2", bufs=1))
        # W1 weight pool opened BEFORE LN2's transient pools (same
        # region-overlap stall as the QKV weight pool at LN1)
        w1pre_stack = ExitStack()
        w1_pool = w1pre_stack.enter_context(tc.tile_pool(name="w1s",
                                                        bufs=20))
        ln2_stack = ExitStack()
        ln2_sq = ln2_stack.enter_context(tc.tile_pool(name="sql2", bufs=3))
        ln2_tm = ln2_stack.enter_context(tc.tile_pool(name="tml2", bufs=3))
        ln2_ve = ln2_stack.enter_context(tc.tile_pool(name="vel2", bufs=4))
        xm = [xm_pool.tile([128, TOK], FP32R, tag=f"xm{ct}", name=f"xm{ct}")
              for ct in range(CT)]
        with ExitStack() as ph:
            pj_ps = ln2_stack.enter_context(
                tc.tile_pool(name="pjps", bufs=8, space="PSUM"))
            for og in range(4):
                pss = {}
                for nh in range(2):
                    for ot in range(4):
                        pss[(nh, ot)] = pj_ps.tile(
                            [128, NH], FP32, tag="pj",
                            name=f"pspj{og}_{nh}_{ot}")
                for kp in range(KP):
                    wt = wp_pool.tile([128, 2, 512], FP8, tag="wp")
                    c0 = (og * KP + kp) * 1024
                    nc.sync.dma_start(out=wt, in_=wp8[:, c0:c0 + 1024])
                    for nh in range(2):
                        t0 = nh * NH
                        for ot in range(4):
                            nc.tensor.matmul(
                                out=pss[(nh, ot)],
                                lhsT=wt[:, :, ot * 128:(ot + 1) * 128],
                                rhs=o_sb[kp][:, :, t0:t0 + NH],
                                perf_mode=DR,
                                start=(kp == 0), stop=(kp == KP - 1))
                for nh in range(2):
                    c0 = nh * NH
                    for ot in range(4):
                        go = og * 4 + ot
                        xr = xr_pool.tile([128, NH], FP32, tag="xr")
                        nc.sync.dma_start(
                            out=xr,
                            in_=xrT[go * 128:(go + 1) * 128, c0:c0 + NH])
                        nc.vector.scalar_tensor_tensor(
                            out=xm[go][:, c0:c0 + NH],
                            in0=pss[(nh, ot)],
                            scalar=EV_PJ,
                            in1=xr, op0=ALU.mult, op1=ALU.add)

        pjs_stack.close()

        # ============ LN2 (+ fold b2 into x_mid in place) ============
        xt2w = [xt2_pool.tile([128, TOK], BF16, tag=f"x2{ct}",
                              name=f"xt2w{ct}") for ct in range(CT)]

        def m_loader(ct, pass_i):
            return xm[ct]

        layernorm_wide("l2", m_loader, g2s, be2s,
                       lambda ct, j: xt2w[ct][:, j * NH:(j + 1) * NH],
                       in_fp32r=True,
                       pools=(pj_ps, ln2_sq, ln2_tm, ln2_ve))
        ln2_stack.close()
        xt2 = {0: [t[:, 0:NH] for t in xt2w], 1: [t[:, NH:TOK] for t in xt2w]}
        for ct in range(CT):
            nc.vector.tensor_scalar_add(
                out=xm[ct],
                in0=xm[ct].bitcast(FP32),
                scalar1=b2s[:, ct:ct + 1])

        # ================= MLP (bf16) =================
        with ExitStack() as ph:
            w2_pool = ph.enter_context(tc.tile_pool(name="w2s", bufs=CH + 1))
            h1_pool = ph.enter_context(
                tc.tile_pool(name="h1", bufs=2 * CH + 2))
            mlp_ps = ph.enter_context(
                tc.tile_pool(name="mlpps", bufs=8, space="PSUM"))
            for fc in range(FT // CH):
                h1 = {}
                for half in range(CH // 4):
                    f0 = fc * CH + half * 4
                    w1ts = []
                    for ct in range(CT):
                        wt = w1_pool.tile([128, 512], BF16, tag="w1")
                        nc.gpsimd.dma_start(
                            out=wt,
                            in_=w1T[ct * 128:(ct + 1) * 128,
                                    f0 * 128:(f0 + 4) * 128])
                        w1ts.append(wt)
                    # nh-major: the nh=0 groups consume only clip0 of LN2's
                    # output, which is emitted first
                    for nh in range(2):
                        psh = {fi: mlp_ps.tile(
                            [128, NH], FP32, tag="mlp",
                            name=f"psh{fc}_{half}_{nh}_{fi}")
                            for fi in range(4)}
                        for ct in range(CT):
                            for fi in range(4):
                                nc.tensor.matmul(
                                    out=psh[fi],
                                    lhsT=w1ts[ct][:,
                                                  fi * 128:(fi + 1) * 128],
                                    rhs=xt2[nh][ct],
                                    start=(ct == 0), stop=(ct == CT - 1))
                        for fi in range(4):
                            f = f0 + fi
                            ht = h1_pool.tile([128, NH], BF16, tag="h1")
                            nc.scalar.activation(out=ht, in_=psh[fi],
                                                 func=AF.Gelu,
                                                 bias=b1s[:, f:f + 1])
                            h1[(nh, half * 4 + fi)] = ht
                for qd in range(4):
                    w2ts = []
                    for fi in range(CH):
                        f = fc * CH + fi
                        wt = w2_pool.tile([128, 512], BF16, tag="w2")
                        nc.gpsimd.dma_start(
                            out=wt,
                            in_=w2T[f * 128:(f + 1) * 128,
                                    qd * 512:(qd + 1) * 512])
                        w2ts.append(wt)
                    for nh in range(2):
                        c0 = nh * NH
                        pss = [mlp_ps.tile([128, NH], FP32, tag="mlp",
                                           name=f"psw2_{fc}_{qd}_{nh}_{i}")
                               for i in range(4)]
                        for fi in range(CH):
                            for ot in range(4):
                                nc.tensor.matmul(
                                    out=pss[ot],
                                    lhsT=w2ts[fi][:, ot * 128:(ot + 1) * 128],
                                    rhs=h1[(nh, fi)],
                                    start=(fi == 0), stop=(fi == CH - 1))
                        for ot in range(4):
                            go = qd * 4 + ot
                            nc.vector.tensor_add(
                                out=xm[go][:, c0:c0 + NH],
                                in0=xm[go][:, c0:c0 + NH].bitcast(FP32),
                                in1=pss[ot])
        w1pre_stack.close()
        xt2_stack.close()

        # ================= Output =================
        for ct in range(CT):
            nc.sync.dma_start(
                out=outT[ct * 128:(ct + 1) * 128, :],
                in_=xm[ct].bitcast(FP32))
        xmid_stack.close()

    _orig_tables = _patch_act_tables()
    try:
        nc.compile()
    finally:
        import concourse.bacc as _bacc
        _bacc.get_activation_tables = _orig_tables
    return nc


_NC = None


def _get_nc():
    global _NC
    if _NC is None:
        _NC = build()
    return _NC


def _dr_pack(WT, groups, gwidth):
    """[K, M] -> [128, (K//256)*2*M] DoubleRow layout, phase-ordered.

    M is consumed in `groups` blocks of `gwidth` columns; output column
    order is (g, kp, i, m'): tile (g, kp) = cols
    [(g*KP+kp)*2*gwidth : +2*gwidth] viewed as [128, 2, gwidth]."""
    import ml_dtypes
    K, M = WT.shape
    assert groups * gwidth == M and K % 256 == 0
    a = (np.asarray(WT, np.float32) * S_W).astype(ml_dtypes.float8_e4m3)
    a = a.reshape(K // 256, 2, 128, groups, gwidth)     # [kp, i, p, g, m']
    a = a.transpose(2, 3, 0, 1, 4)                      # [p, g, kp, i, m']
    return np.ascontiguousarray(a.reshape(128, -1))


def _prep_shared(Wqkv, Wproj, bproj, gamma1, beta1, gamma2, beta2, W1, b1,
                 W2, b2):
    import ml_dtypes

    def f32(a):
        return np.ascontiguousarray(np.asarray(a, dtype=np.float32))

    def bf16(a):
        return np.ascontiguousarray(
            np.asarray(a, dtype=np.float32).astype(ml_dtypes.bfloat16))

    Wqkv = np.asarray(Wqkv, np.float32)
    return {
        "wq8": _dr_pack(Wqkv[0:DIM].T, 2, 1024),
        "wk8": _dr_pack(Wqkv[DIM:2 * DIM].T, 4, 512),
        "wv8": _dr_pack(Wqkv[2 * DIM:3 * DIM].T, 4, 512),
        "wp8": _dr_pack(np.asarray(Wproj, np.float32).T, 4, 512),
        "w1T": bf16(np.asarray(W1).T),
        "w2T": bf16(np.asarray(W2).T),
        "g1v": f32(np.asarray(gamma1).reshape(CT, 128).T * S_X),
        "be1v": f32(np.asarray(beta1).reshape(CT, 128).T * S_X),
        "g2v": f32(np.asarray(gamma2).reshape(CT, 128).T),
        "be2v": f32(np.asarray(beta2).reshape(CT, 128).T),
        "b1v": f32(np.asarray(b1).reshape(FT, 128).T),
        "b2v": f32(np.asarray(b2).reshape(CT, 128).T),
        "_bproj": f32(bproj),
    }


def build_in_maps(x, gamma1, beta1, Wqkv, Wproj, bproj, gamma2, beta2, W1,
                  b1, W2, b2):
    import ml_dtypes
    x = np.asarray(x, dtype=np.float32)          # [8, 2, 512, 2048]
    shared = _prep_shared(Wqkv, Wproj, bproj, gamma1, beta1, gamma2, beta2,
                          W1, b1, W2, b2)
    bp = shared.pop("_bproj")
    in_maps = []
    for i in range(N_CORES):
        xt = np.ascontiguousarray(x[i].reshape(TOK, DIM).T)
        m = {"xrT": xt + bp[:, None],
             "xTbf": np.ascontiguousarray(xt.astype(ml_dtypes.bfloat16))}
        m.update(shared)
        in_maps.append(m)
    return in_maps


def kernel(x, gamma1, beta1, Wqkv, Wproj, bproj, gamma2, beta2, W1, b1, W2,
           b2):
    nc = _get_nc()
    in_maps = build_in_maps(x, gamma1, beta1, Wqkv, Wproj, bproj, gamma2,
                            beta2, W1, b1, W2, b2)
    res = run_bass_kernel_spmd(nc, in_maps, core_ids=list(range(N_CORES)))
    out = np.stack([
        np.ascontiguousarray(res.results[i]["out"].T).reshape(2, NH, DIM)
        for i in range(N_CORES)
    ])
    return out
